# revision 1
# baseline (speedup 1.0000x reference)
"""Multi-head causal attention (B=4, T=2048, C=1024, H=16) on 8 trn2 cores.

Sharding: data-parallel over batch (4) x sequence-parallel over causal query
blocks (2), zig-zag balanced so all 8 cores run one identical program:
  core = 2*b + half;  half 0 gets query blocks [0,2,4,6,9,11,13,15],
  half 1 gets [1,3,5,7,8,10,12,14].  Slot s (0..7) processes J(s)=2s+2 key
  blocks; causal boundary handled by per-core input masks on the last two.
Each core writes a disjoint [1024, 1024] slice of the output; the host
scatters slices back and adds the (v/o-bias) correction  bo + bv @ Wo.T.
"""

import numpy as np
import ml_dtypes

import concourse.bass as bass
import concourse.mybir as mybir
import concourse.tile as tile
from concourse import bacc
from concourse.bass import ts
from concourse.bass_utils import run_bass_kernel_spmd

B, T, C, H, DK = 4, 2048, 1024, 16, 64
P = 128
NB = T // P          # 16 key blocks
SLOTS = 8            # query blocks per core
SCALE = 1.0 / np.sqrt(DK)
BF16 = mybir.dt.bfloat16
F32 = mybir.dt.float32
F32R = mybir.dt.float32r
EXP = mybir.ActivationFunctionType.Exp

QBLKS = [
    [0, 2, 4, 6, 9, 11, 13, 15],
    [1, 3, 5, 7, 8, 10, 12, 14],
]

_cache = {}


def _build():
    nc = bacc.Bacc("TRN2", target_bir_lowering=False, debug=False)

    xT = nc.dram_tensor("xT", [C, T], BF16, kind="ExternalInput").ap()
    xTq = nc.dram_tensor("xTq", [C, SLOTS * P], BF16, kind="ExternalInput").ap()
    wqT = nc.dram_tensor("wqT", [C, C], BF16, kind="ExternalInput").ap()
    wkT = nc.dram_tensor("wkT", [C, C], BF16, kind="ExternalInput").ap()
    wvT = nc.dram_tensor("wvT", [C, C], BF16, kind="ExternalInput").ap()
    woT = nc.dram_tensor("woT", [C, C], BF16, kind="ExternalInput").ap()
    bq = nc.dram_tensor("bq", [P, C // P], F32, kind="ExternalInput").ap()
    bk = nc.dram_tensor("bk", [P, C // P], F32, kind="ExternalInput").ap()
    masks = nc.dram_tensor("masks", [SLOTS, 2, P, P], BF16, kind="ExternalInput").ap()
    ident = nc.dram_tensor("ident", [P, P], BF16, kind="ExternalInput").ap()
    y = nc.dram_tensor("y", [SLOTS * P, C], F32, kind="ExternalOutput").ap()

    CB = C // P  # 8 column blocks of the channel dim

    with tile.TileContext(nc) as tc:
        with (
            tc.tile_pool(name="const", bufs=1) as cpool,
            tc.tile_pool(name="attn", bufs=1) as apool,
        ):
            # tiles allocated here; DMA emission deferred so the small
            # Q-phase loads head the gpsimd queue (fast PE start)
            masks_sb = cpool.tile([P, SLOTS, 2, P], BF16)
            ident_sb = cpool.tile([P, P], BF16)
            bq_sb = cpool.tile([P, CB], F32)
            bk_sb = cpool.tile([P, CB], F32)

            attn_out = apool.tile([P, SLOTS, C], BF16)

            with tc.tile_pool(name="qkv", bufs=1) as qkv:
                qT = qkv.tile([P, CB, SLOTS * P], BF16)
                kT = qkv.tile([P, CB, T], BF16)
                v = qkv.tile([P, NB, H * (DK + 1)], BF16)
                vg = v[:].rearrange("p t (h e) -> p t h e", e=DK + 1)
                nc.vector.memset(vg[:, :, :, DK : DK + 1], 1.0)

                # ---- phase 1: q/k/v projections (fp32r matmuls) ----
                # Each weight is DMA'd exactly once, split into two half
                # tiles (kb 0-3 / 4-7) so the next projection's first half
                # prefetches while the current one finishes (keeps PE warm).
                with (
                    tc.tile_pool(name="xt", bufs=1) as xt_pool,
                    tc.tile_pool(name="wres", bufs=1) as wres,
                    tc.tile_pool(name="xq", bufs=3) as xq_pool,
                    tc.tile_pool(name="pacc", bufs=1, space="PSUM") as pacc,
                ):
                    xT_sb = xt_pool.tile([P, CB, T], BF16)

                    def load_weight(src):
                        halves = []
                        for i, tag in enumerate(("wA", "wB")):
                            w_t = wres.tile([P, 4, C], BF16, tag=tag, name=tag)
                            nc.gpsimd.dma_start(
                                w_t[:],
                                src.rearrange("(ko p) n -> p ko n", p=P)[
                                    :, 4 * i : 4 * i + 4, :
                                ],
                            )
                            halves.append(w_t)
                        return lambda kb: halves[kb // 4][:, kb % 4, :]

                    # Q projection: qT[c_out, tq]
                    nc.gpsimd.dma_start(bq_sb[:], bq[:])
                    nc.gpsimd.dma_start(bk_sb[:], bk[:])
                    wq_at = load_weight(wqT)
                    for nch in range(2):
                        acc = [
                            pacc.tile([P, 512], F32, tag=f"acc{cb}", name=f"acc{cb}") for cb in range(CB)
                        ]
                        for kb in range(CB):
                            xq_ch = xq_pool.tile([P, 512], BF16, tag="xq")
                            nc.gpsimd.dma_start(
                                xq_ch[:],
                                xTq.rearrange("(ko p) t -> p ko t", p=P)[
                                    :, kb, ts(nch, 512)
                                ],
                            )
                            for cb in range(CB):
                                nc.tensor.matmul(
                                    acc[cb][:],
                                    wq_at(kb)[:, ts(cb, P)],
                                    xq_ch[:],
                                    start=(kb == 0),
                                    stop=(kb == CB - 1),
                                )
                        for cb in range(CB):
                            nc.vector.tensor_scalar_add(
                                qT[:, cb, ts(nch, 512)], acc[cb][:], bq_sb[:, cb : cb + 1]
                            )

                    # deferred big/const loads: emitted after Q's DMAs so the
                    # PE starts within ~8us; they overlap Q compute
                    nc.gpsimd.dma_start(
                        xT_sb[:], xT.rearrange("(ko p) t -> p ko t", p=P)
                    )
                    nc.gpsimd.dma_start(
                        masks_sb[:], masks[:].rearrange("s t p q -> p s t q")
                    )
                    nc.gpsimd.dma_start(ident_sb[:], ident[:])

                    # K projection: kT[c_out, t]
                    wk_at = load_weight(wkT)
                    for nch in range(4):
                        acc = [
                            pacc.tile([P, 512], F32, tag=f"acc{cb}", name=f"acc{cb}") for cb in range(CB)
                        ]
                        for kb in range(CB):
                            for cb in range(CB):
                                nc.tensor.matmul(
                                    acc[cb][:],
                                    wk_at(kb)[:, ts(cb, P)],
                                    xT_sb[:, kb, ts(nch, 512)],
                                    start=(kb == 0),
                                    stop=(kb == CB - 1),
                                )
                        for cb in range(CB):
                            nc.vector.tensor_scalar_add(
                                kT[:, cb, ts(nch, 512)], acc[cb][:], bk_sb[:, cb : cb + 1]
                            )

                    # V projection: v[t, d] natural layout, head-grouped with
                    # a ones column per head (free softmax denominator).
                    # Token blocks in groups of 4 so the 8 live accumulators
                    # fit PSUM.
                    wv_at = load_weight(wvT)
                    for tbg in range(NB // 4):
                        acc = [
                            pacc.tile([P, 512], F32, tag=f"acc{i}", name=f"vacc{i}")
                            for i in range(8)
                        ]
                        for kb in range(CB):
                            for ti in range(4):
                                tb = tbg * 4 + ti
                                for dch in range(2):
                                    nc.tensor.matmul(
                                        acc[ti * 2 + dch][:],
                                        xT_sb[:, kb, ts(tb, P)],
                                        wv_at(kb)[:, ts(dch, 512)],
                                        start=(kb == 0),
                                        stop=(kb == CB - 1),
                                    )
                        for ti in range(4):
                            tb = tbg * 4 + ti
                            for dch in range(2):
                                nc.vector.tensor_copy(
                                    vg[:, tb, dch * 8 : (dch + 1) * 8, 0:DK],
                                    acc[ti * 2 + dch][:].rearrange(
                                        "p (h e) -> p h e", e=DK
                                    ),
                                )

                # ---- phase 2: attention per head ----
                with (
                    tc.tile_pool(name="expS", bufs=3) as spool,
                    tc.tile_pool(name="small", bufs=8) as small,
                    tc.tile_pool(name="ps_s", bufs=3, space="PSUM") as ps_s,
                    tc.tile_pool(name="ps_o", bufs=2, space="PSUM") as ps_o,
                ):
                    for h in range(H):
                        hp = (h % 2) * DK
                        cbh = h // 2
                        expS = spool.tile([P, NB, SLOTS * P], BF16, tag="expS")
                        for jb in range(NB):
                            smin = jb // 2
                            q0 = smin * P
                            if jb >= 8 and jb % 2 == 1:
                                continue  # merged into even sibling below
                            pss = ps_s.tile([P, SLOTS * P], F32, tag="ps_s")
                            if jb >= 8:
                                # pair (jb, jb+1): same q-range, <=512 wide;
                                # two bank-aligned matmuls, ONE exp drains both
                                w = SLOTS * P - q0
                                for i in range(2):
                                    nc.tensor.matmul(
                                        pss[:, 512 * i : 512 * i + w],
                                        kT[hp : hp + DK, cbh, ts(jb + i, P)],
                                        qT[hp : hp + DK, cbh, q0:],
                                        start=True,
                                        stop=True,
                                    )
                                nc.scalar.activation(
                                    expS[:, jb : jb + 2, q0:],
                                    pss[:].rearrange("p (t c) -> p t c", t=2)[
                                        :, :, 0:w
                                    ],
                                    EXP,
                                    scale=float(SCALE),
                                )
                                jbs = (jb, jb + 1)
                            else:
                                cuts = sorted({q0, 512, SLOTS * P})
                                for a, b in zip(cuts, cuts[1:]):
                                    if a < q0:
                                        continue
                                    nc.tensor.matmul(
                                        pss[:, a:b],
                                        kT[hp : hp + DK, cbh, ts(jb, P)],
                                        qT[hp : hp + DK, cbh, a:b],
                                        start=True,
                                        stop=True,
                                    )
                                nc.scalar.activation(
                                    expS[:, jb, q0:], pss[:, q0:], EXP,
                                    scale=float(SCALE),
                                )
                                jbs = (jb,)
                            # causal boundary: slot j//2 sees j as one of its
                            # last-two key blocks; mask multiplies after exp.
                            for j in jbs:
                                sm = j // 2
                                nc.vector.tensor_mul(
                                    expS[:, j, ts(sm, P)],
                                    expS[:, j, ts(sm, P)],
                                    masks_sb[:, sm, j % 2, :],
                                )
                        for s in range(SLOTS):
                            J = 2 * s + 2
                            pso = ps_o.tile([P, DK + 1], F32, tag="ps_o")
                            for jb in range(J):
                                nc.tensor.matmul(
                                    pso[:],
                                    expS[:, jb, ts(s, P)],
                                    v[:, jb, h * (DK + 1) : (h + 1) * (DK + 1)],
                                    start=(jb == 0),
                                    stop=(jb == J - 1),
                                )
                            rec = small.tile([P, 1], F32, tag="rec")
                            nc.vector.reciprocal(rec[:], pso[:, DK : DK + 1])
                            nc.vector.tensor_scalar_mul(
                                attn_out[:, s, h * DK : (h + 1) * DK],
                                pso[:, 0:DK],
                                rec[:],
                            )

            # ---- phase 3: transpose + output projection ----
            with (
                tc.tile_pool(name="out", bufs=1) as opool,
                tc.tile_pool(name="ps_t", bufs=4, space="PSUM") as ps_t,
                tc.tile_pool(name="ps_y", bufs=2, space="PSUM") as ps_y,
            ):
                aT = opool.tile([P, CB, SLOTS * P], BF16)
                for cb in range(CB):
                    for s in range(SLOTS):
                        pst = ps_t.tile([P, P], BF16, tag="ps_t")
                        nc.tensor.transpose(
                            pst[:], attn_out[:, s, ts(cb, P)], ident_sb[:]
                        )
                        nc.vector.tensor_copy(aT[:, cb, ts(s, P)], pst[:])

                woT_sb = opool.tile([P, CB, C], BF16)
                nc.gpsimd.dma_start(
                    woT_sb[:], woT.rearrange("(ko p) n -> p ko n", p=P)
                )
                y_sb = opool.tile([P, SLOTS, C], F32)
                for tb in range(SLOTS):
                    for nch in range(2):
                        psy = ps_y.tile([P, 512], F32, tag="ps_y")
                        for cbk in range(CB):
                            nc.tensor.matmul(
                                psy[:],
                                aT[:, cbk, ts(tb, P)],
                                woT_sb[:, cbk, ts(nch, 512)],
                                start=(cbk == 0),
                                stop=(cbk == CB - 1),
                            )
                        nc.vector.tensor_copy(y_sb[:, tb, ts(nch, 512)], psy[:])
                    # per-block writeback overlaps remaining Y matmuls;
                    # only the last 512KB slice is an exposed tail
                    nc.gpsimd.dma_start(
                        y.rearrange("(tb p) c -> p tb c", p=P)[:, tb, :],
                        y_sb[:, tb, :],
                    )

    nc.compile()
    return nc


def _host_inputs(x, mask, Wq, bq_v, Wk, bk_v, Wv, bv_v, Wo, bo_v):
    """Per-core input maps + the host-side output bias correction."""
    f32 = np.float32
    bf16 = ml_dtypes.bfloat16
    wqT = np.ascontiguousarray(np.asarray(Wq, f32).T).astype(bf16)
    wkT = np.ascontiguousarray(np.asarray(Wk, f32).T).astype(bf16)
    wvT = np.ascontiguousarray(np.asarray(Wv, f32).T).astype(bf16)
    woT = np.ascontiguousarray(np.asarray(Wo, f32).T).astype(bf16)
    bq_p = np.ascontiguousarray(np.asarray(bq_v, f32).reshape(C // P, P).T)
    bk_p = np.ascontiguousarray(np.asarray(bk_v, f32).reshape(C // P, P).T)
    identity = np.eye(P, dtype=f32).astype(bf16)
    # exact v/o bias fold: softmax rows sum to 1, so v+bv adds bv to attn out
    bo_eff = (np.asarray(bo_v, f32) + np.asarray(bv_v, f32) @ np.asarray(Wo, f32).T)

    # per-half causal boundary masks for the last two key blocks of each slot
    mask_half = []
    tri = np.tril(np.ones((P, P), f32)).T  # [j, i] = 1 where j <= i
    for half in range(2):
        m = np.zeros((SLOTS, 2, P, P), f32)
        for s in range(SLOTS):
            g = QBLKS[half][s]
            for idx, jb in enumerate((2 * s, 2 * s + 1)):
                if jb < g:
                    m[s, idx] = 1.0
                elif jb == g:
                    m[s, idx] = tri
        mask_half.append(m.astype(bf16))

    xn = np.asarray(x, f32)
    in_maps = []
    for core in range(8):
        b, half = divmod(core, 2)
        xT = np.ascontiguousarray(xn[b].T).astype(bf16)
        qtok = np.concatenate([np.arange(g * P, (g + 1) * P) for g in QBLKS[half]])
        xTq = np.ascontiguousarray(xn[b][qtok].T).astype(bf16)
        in_maps.append(
            {
                "xT": xT,
                "xTq": xTq,
                "wqT": wqT,
                "wkT": wkT,
                "wvT": wvT,
                "woT": woT,
                "bq": bq_p,
                "bk": bk_p,
                "masks": mask_half[half],
                "ident": identity,
            }
        )
    return in_maps, bo_eff


def _run(inputs, trace=False):
    if "nc" not in _cache:
        _cache["nc"] = _build()
    nc = _cache["nc"]
    in_maps, bo_eff = _host_inputs(
        inputs["x"], inputs["mask"],
        inputs["Wq"], inputs["bq"], inputs["Wk"], inputs["bk"],
        inputs["Wv"], inputs["bv"], inputs["Wo"], inputs["bo"],
    )
    res = run_bass_kernel_spmd(nc, in_maps, list(range(8)), trace=trace)
    out = np.empty((B, T, C), np.float32)
    for core in range(8):
        b, half = divmod(core, 2)
        yc = res.results[core]["y"]
        for s, g in enumerate(QBLKS[half]):
            out[b, g * P : (g + 1) * P] = yc[s * P : (s + 1) * P]
    out += bo_eff
    return out, res


def kernel(**inputs):
    out, _ = _run(inputs, trace=False)
    return out



# revision 8
# speedup vs baseline: 1.0331x; 1.0331x over previous
"""Multi-head causal attention (B=4, T=2048, C=1024, H=16) on 8 trn2 cores.

Sharding: data-parallel over batch (4) x sequence-parallel over causal query
blocks (2), zig-zag balanced so all 8 cores run one identical program:
  core = 2*b + half;  half 0 gets query blocks [0,2,4,6,9,11,13,15],
  half 1 gets [1,3,5,7,8,10,12,14].  Slot s (0..7) processes J(s)=2s+2 key
  blocks; causal boundary handled by per-core input masks on the last two.
Each core writes a disjoint [1024, 1024] slice of the output; the host
scatters slices back and adds the (v/o-bias) correction  bo + bv @ Wo.T.
"""

import numpy as np
import ml_dtypes

import concourse.bass as bass
import concourse.mybir as mybir
import concourse.tile as tile
from concourse import bacc
from concourse.bass import ts
from concourse.bass_utils import run_bass_kernel_spmd

B, T, C, H, DK = 4, 2048, 1024, 16, 64
P = 128
NB = T // P          # 16 key blocks
SLOTS = 8            # query blocks per core
SCALE = 1.0 / np.sqrt(DK)
BF16 = mybir.dt.bfloat16
F32 = mybir.dt.float32
F32R = mybir.dt.float32r
EXP = mybir.ActivationFunctionType.Exp

QBLKS = [
    [0, 2, 4, 6, 9, 11, 13, 15],
    [1, 3, 5, 7, 8, 10, 12, 14],
]

_cache = {}


def _build():
    nc = bacc.Bacc("TRN2", target_bir_lowering=False, debug=False)

    xT = nc.dram_tensor("xT", [C, T], BF16, kind="ExternalInput").ap()
    xTq = nc.dram_tensor("xTq", [C, SLOTS * P], BF16, kind="ExternalInput").ap()
    wqT = nc.dram_tensor("wqT", [C, C], BF16, kind="ExternalInput").ap()
    wkT = nc.dram_tensor("wkT", [C, C], BF16, kind="ExternalInput").ap()
    wvT = nc.dram_tensor("wvT", [C, C], BF16, kind="ExternalInput").ap()
    woT = nc.dram_tensor("woT", [C, C], BF16, kind="ExternalInput").ap()
    bq = nc.dram_tensor("bq", [P, C // P], F32, kind="ExternalInput").ap()
    bk = nc.dram_tensor("bk", [P, C // P], F32, kind="ExternalInput").ap()
    masks = nc.dram_tensor("masks", [SLOTS, 2, P, P], BF16, kind="ExternalInput").ap()
    ident = nc.dram_tensor("ident", [P, P], BF16, kind="ExternalInput").ap()
    y = nc.dram_tensor("y", [SLOTS * P, C], F32, kind="ExternalOutput").ap()

    CB = C // P  # 8 column blocks of the channel dim

    with tile.TileContext(nc) as tc:
        with (
            tc.tile_pool(name="const", bufs=1) as cpool,
            tc.tile_pool(name="attn", bufs=1) as apool,
        ):
            # tiles allocated here; DMA emission deferred so the small
            # Q-phase loads head the gpsimd queue (fast PE start)
            masks_sb = cpool.tile([P, SLOTS, 2, P], BF16)
            ident_sb = cpool.tile([P, P], BF16)
            bq_sb = cpool.tile([P, CB], F32)
            bk_sb = cpool.tile([P, CB], F32)

            attn_out = apool.tile([P, SLOTS, C], BF16)
            woT_sb = apool.tile([P, C // P, C], BF16)

            with tc.tile_pool(name="qkv", bufs=1) as qkv:
                qT = qkv.tile([P, CB, SLOTS * P], BF16)
                kT = qkv.tile([P, CB, T], BF16)
                v = qkv.tile([P, NB, H * (DK + 1)], BF16)
                vg = v[:].rearrange("p t (h e) -> p t h e", e=DK + 1)
                nc.vector.memset(vg[:, :, :, DK : DK + 1], 1.0)

                # ---- phase 1: q/k/v projections (fp32r matmuls) ----
                # Each weight is DMA'd exactly once, split into two half
                # tiles (kb 0-3 / 4-7) so the next projection's first half
                # prefetches while the current one finishes (keeps PE warm).
                with (
                    tc.tile_pool(name="xt", bufs=1) as xt_pool,
                    tc.tile_pool(name="wres", bufs=1) as wres,
                    tc.tile_pool(name="xq", bufs=1) as xq_pool,
                    tc.tile_pool(name="pacc", bufs=1, space="PSUM") as pacc,
                ):
                    xT_sb = xt_pool.tile([P, CB, T], BF16)

                    def load_weight(src):
                        halves = []
                        for i, tag in enumerate(("wA", "wB")):
                            w_t = wres.tile([P, 4, C], BF16, tag=tag, name=tag)
                            nc.gpsimd.dma_start(
                                w_t[:],
                                src.rearrange("(ko p) n -> p ko n", p=P)[
                                    :, 4 * i : 4 * i + 4, :
                                ],
                            )
                            halves.append(w_t)
                        return lambda kb: halves[kb // 4][:, kb % 4, :]

                    # Q projection: qT[c_out, tq].  xTq loaded as one resident
                    # tile via per-kb DMAs so the first matmul only waits on
                    # column block 0 (~256KB) and Q never stalls mid-flight
                    # (a >3.4us PE gap here re-throttles the HAM clock gate).
                    nc.gpsimd.dma_start(bq_sb[:], bq[:])
                    nc.gpsimd.dma_start(bk_sb[:], bk[:])
                    xq_sb = xq_pool.tile([P, CB, SLOTS * P], BF16)
                    xTq_r = xTq.rearrange("(ko p) t -> p ko t", p=P)
                    nc.gpsimd.dma_start(xq_sb[:, 0, :], xTq_r[:, 0, :])
                    wq_at = load_weight(wqT)
                    for kb in range(1, CB):
                        nc.gpsimd.dma_start(xq_sb[:, kb, :], xTq_r[:, kb, :])
                    for nch in range(2):
                        acc = [
                            pacc.tile([P, 512], F32, tag=f"acc{cb}", name=f"acc{cb}") for cb in range(CB)
                        ]
                        for kb in range(CB):
                            for cb in range(CB):
                                nc.tensor.matmul(
                                    acc[cb][:],
                                    wq_at(kb)[:, ts(cb, P)],
                                    xq_sb[:, kb, ts(nch, 512)],
                                    start=(kb == 0),
                                    stop=(kb == CB - 1),
                                )
                        for cb in range(CB):
                            nc.vector.tensor_scalar_add(
                                qT[:, cb, ts(nch, 512)], acc[cb][:], bq_sb[:, cb : cb + 1]
                            )

                    # deferred big/const loads: emitted after Q's DMAs so the
                    # PE starts within ~8us; they overlap Q compute
                    nc.gpsimd.dma_start(
                        xT_sb[:], xT.rearrange("(ko p) t -> p ko t", p=P)
                    )
                    nc.gpsimd.dma_start(
                        masks_sb[:], masks[:].rearrange("s t p q -> p s t q")
                    )
                    nc.gpsimd.dma_start(ident_sb[:], ident[:])

                    # K projection: kT[c_out, t]
                    wk_at = load_weight(wkT)
                    for nch in range(4):
                        acc = [
                            pacc.tile([P, 512], F32, tag=f"acc{cb}", name=f"acc{cb}") for cb in range(CB)
                        ]
                        for kb in range(CB):
                            for cb in range(CB):
                                nc.tensor.matmul(
                                    acc[cb][:],
                                    wk_at(kb)[:, ts(cb, P)],
                                    xT_sb[:, kb, ts(nch, 512)],
                                    start=(kb == 0),
                                    stop=(kb == CB - 1),
                                )
                        for cb in range(CB):
                            nc.vector.tensor_scalar_add(
                                kT[:, cb, ts(nch, 512)], acc[cb][:], bk_sb[:, cb : cb + 1]
                            )

                    # V projection: v[t, d] natural layout, head-grouped with
                    # a ones column per head (free softmax denominator).
                    # Token blocks in groups of 4 so the 8 live accumulators
                    # fit PSUM.
                    # Groups of 3 token blocks (6 live accumulators) leave 2
                    # PSUM banks free so phase 2's first score tiles allocate
                    # without waiting on the V drain (no PE gap at the
                    # boundary -> HAM stays at full clock into attention).
                    wv_at = load_weight(wvT)
                    tb0 = 0
                    for gsz in (3, 3, 3, 3, 3, 1):
                        acc = [
                            pacc.tile([P, 512], F32, tag=f"acc{i}", name=f"vacc{i}")
                            for i in range(2 * gsz)
                        ]
                        for kb in range(CB):
                            for ti in range(gsz):
                                tb = tb0 + ti
                                for dch in range(2):
                                    nc.tensor.matmul(
                                        acc[ti * 2 + dch][:],
                                        xT_sb[:, kb, ts(tb, P)],
                                        wv_at(kb)[:, ts(dch, 512)],
                                        start=(kb == 0),
                                        stop=(kb == CB - 1),
                                    )
                        for ti in range(gsz):
                            tb = tb0 + ti
                            for dch in range(2):
                                nc.vector.tensor_copy(
                                    vg[:, tb, dch * 8 : (dch + 1) * 8, 0:DK],
                                    acc[ti * 2 + dch][:].rearrange(
                                        "p (h e) -> p h e", e=DK
                                    ),
                                )
                        tb0 += gsz

                # ---- phase 2: attention per head ----
                with (
                    tc.tile_pool(name="expS", bufs=2) as spool,
                    tc.tile_pool(name="small", bufs=8) as small,
                    tc.tile_pool(name="ps_s", bufs=3, space="PSUM") as ps_s,
                    tc.tile_pool(name="ps_o", bufs=2, space="PSUM") as ps_o,
                ):
                    for h in range(H):
                        if h == 4:
                            # prefetch Wo during attention so the output
                            # projection never waits on HBM
                            nc.gpsimd.dma_start(
                                woT_sb[:], woT.rearrange("(ko p) n -> p ko n", p=P)
                            )
                        hp = (h % 2) * DK
                        cbh = h // 2
                        expS = spool.tile([P, NB, SLOTS * P], BF16, tag="expS")
                        for jb in range(NB):
                            smin = jb // 2
                            q0 = smin * P
                            if jb >= 8 and jb % 2 == 1:
                                continue  # merged into even sibling below
                            pss = ps_s.tile([P, SLOTS * P], F32, tag="ps_s")
                            if jb >= 8:
                                # pair (jb, jb+1): same q-range, <=512 wide;
                                # two bank-aligned matmuls, ONE exp drains both
                                w = SLOTS * P - q0
                                for i in range(2):
                                    nc.tensor.matmul(
                                        pss[:, 512 * i : 512 * i + w],
                                        kT[hp : hp + DK, cbh, ts(jb + i, P)],
                                        qT[hp : hp + DK, cbh, q0:],
                                        start=True,
                                        stop=True,
                                    )
                                nc.scalar.activation(
                                    expS[:, jb : jb + 2, q0:],
                                    pss[:].rearrange("p (t c) -> p t c", t=2)[
                                        :, :, 0:w
                                    ],
                                    EXP,
                                    scale=float(SCALE),
                                )
                                jbs = (jb, jb + 1)
                            else:
                                cuts = sorted({q0, 512, SLOTS * P})
                                for a, b in zip(cuts, cuts[1:]):
                                    if a < q0:
                                        continue
                                    nc.tensor.matmul(
                                        pss[:, a:b],
                                        kT[hp : hp + DK, cbh, ts(jb, P)],
                                        qT[hp : hp + DK, cbh, a:b],
                                        start=True,
                                        stop=True,
                                    )
                                nc.scalar.activation(
                                    expS[:, jb, q0:], pss[:, q0:], EXP,
                                    scale=float(SCALE),
                                )
                                jbs = (jb,)
                            # causal boundary: slot j//2 sees j as one of its
                            # last-two key blocks; mask multiplies after exp.
                            for j in jbs:
                                sm = j // 2
                                nc.vector.tensor_mul(
                                    expS[:, j, ts(sm, P)],
                                    expS[:, j, ts(sm, P)],
                                    masks_sb[:, sm, j % 2, :],
                                )
                        for s in range(SLOTS):
                            J = 2 * s + 2
                            pso = ps_o.tile([P, DK + 1], F32, tag="ps_o")
                            for jb in range(J):
                                nc.tensor.matmul(
                                    pso[:],
                                    expS[:, jb, ts(s, P)],
                                    v[:, jb, h * (DK + 1) : (h + 1) * (DK + 1)],
                                    start=(jb == 0),
                                    stop=(jb == J - 1),
                                )
                            rec = small.tile([P, 1], F32, tag="rec")
                            nc.vector.reciprocal(rec[:], pso[:, DK : DK + 1])
                            nc.vector.tensor_scalar_mul(
                                attn_out[:, s, h * DK : (h + 1) * DK],
                                pso[:, 0:DK],
                                rec[:],
                            )

            # ---- phase 3: transpose + output projection ----
            with (
                tc.tile_pool(name="out", bufs=1) as opool,
                tc.tile_pool(name="ps_t", bufs=4, space="PSUM") as ps_t,
                tc.tile_pool(name="ps_y", bufs=2, space="PSUM") as ps_y,
            ):
                aT = opool.tile([P, CB, SLOTS * P], BF16)
                for cb in range(CB):
                    for s in range(SLOTS):
                        pst = ps_t.tile([P, P], BF16, tag="ps_t")
                        nc.tensor.transpose(
                            pst[:], attn_out[:, s, ts(cb, P)], ident_sb[:]
                        )
                        nc.vector.tensor_copy(aT[:, cb, ts(s, P)], pst[:])

                y_sb = opool.tile([P, SLOTS, C], F32)
                for tb in range(SLOTS):
                    for nch in range(2):
                        psy = ps_y.tile([P, 512], F32, tag="ps_y")
                        for cbk in range(CB):
                            nc.tensor.matmul(
                                psy[:],
                                aT[:, cbk, ts(tb, P)],
                                woT_sb[:, cbk, ts(nch, 512)],
                                start=(cbk == 0),
                                stop=(cbk == CB - 1),
                            )
                        nc.vector.tensor_copy(y_sb[:, tb, ts(nch, 512)], psy[:])
                    # per-block writeback overlaps remaining Y matmuls;
                    # only the last 512KB slice is an exposed tail
                    nc.gpsimd.dma_start(
                        y.rearrange("(tb p) c -> p tb c", p=P)[:, tb, :],
                        y_sb[:, tb, :],
                    )

    nc.compile()
    return nc


def _host_inputs(x, mask, Wq, bq_v, Wk, bk_v, Wv, bv_v, Wo, bo_v):
    """Per-core input maps + the host-side output bias correction."""
    f32 = np.float32
    bf16 = ml_dtypes.bfloat16
    wqT = np.ascontiguousarray(np.asarray(Wq, f32).T).astype(bf16)
    wkT = np.ascontiguousarray(np.asarray(Wk, f32).T).astype(bf16)
    wvT = np.ascontiguousarray(np.asarray(Wv, f32).T).astype(bf16)
    woT = np.ascontiguousarray(np.asarray(Wo, f32).T).astype(bf16)
    bq_p = np.ascontiguousarray(np.asarray(bq_v, f32).reshape(C // P, P).T)
    bk_p = np.ascontiguousarray(np.asarray(bk_v, f32).reshape(C // P, P).T)
    identity = np.eye(P, dtype=f32).astype(bf16)
    # exact v/o bias fold: softmax rows sum to 1, so v+bv adds bv to attn out
    bo_eff = (np.asarray(bo_v, f32) + np.asarray(bv_v, f32) @ np.asarray(Wo, f32).T)

    # per-half causal boundary masks for the last two key blocks of each slot
    mask_half = []
    tri = np.tril(np.ones((P, P), f32)).T  # [j, i] = 1 where j <= i
    for half in range(2):
        m = np.zeros((SLOTS, 2, P, P), f32)
        for s in range(SLOTS):
            g = QBLKS[half][s]
            for idx, jb in enumerate((2 * s, 2 * s + 1)):
                if jb < g:
                    m[s, idx] = 1.0
                elif jb == g:
                    m[s, idx] = tri
        mask_half.append(m.astype(bf16))

    xn = np.asarray(x, f32)
    in_maps = []
    for core in range(8):
        b, half = divmod(core, 2)
        xT = np.ascontiguousarray(xn[b].T).astype(bf16)
        qtok = np.concatenate([np.arange(g * P, (g + 1) * P) for g in QBLKS[half]])
        xTq = np.ascontiguousarray(xn[b][qtok].T).astype(bf16)
        in_maps.append(
            {
                "xT": xT,
                "xTq": xTq,
                "wqT": wqT,
                "wkT": wkT,
                "wvT": wvT,
                "woT": woT,
                "bq": bq_p,
                "bk": bk_p,
                "masks": mask_half[half],
                "ident": identity,
            }
        )
    return in_maps, bo_eff


def _run(inputs, trace=False):
    if "nc" not in _cache:
        _cache["nc"] = _build()
    nc = _cache["nc"]
    in_maps, bo_eff = _host_inputs(
        inputs["x"], inputs["mask"],
        inputs["Wq"], inputs["bq"], inputs["Wk"], inputs["bk"],
        inputs["Wv"], inputs["bv"], inputs["Wo"], inputs["bo"],
    )
    res = run_bass_kernel_spmd(nc, in_maps, list(range(8)), trace=trace)
    out = np.empty((B, T, C), np.float32)
    for core in range(8):
        b, half = divmod(core, 2)
        yc = res.results[core]["y"]
        for s, g in enumerate(QBLKS[half]):
            out[b, g * P : (g + 1) * P] = yc[s * P : (s + 1) * P]
    out += bo_eff
    return out, res


def kernel(**inputs):
    out, _ = _run(inputs, trace=False)
    return out



# revision 15
# speedup vs baseline: 1.3020x; 1.2603x over previous
"""Multi-head causal attention (B=4, T=2048, C=1024, H=16) on 8 trn2 cores.

Sharding: data-parallel over batch (4) x sequence-parallel over causal query
blocks (2), zig-zag balanced so all 8 cores run one identical program:
  core = 2*b + half;  half 0 gets query blocks [0,2,4,6,9,11,13,15],
  half 1 gets [1,3,5,7,8,10,12,14].  Slot s (0..7) processes J(s)=2s+2 key
  blocks; causal boundary handled by per-core input masks on the last two.
Each core writes a disjoint [1024, 1024] slice of the output; the host
scatters slices back and adds the (v/o-bias) correction  bo + bv @ Wo.T.
"""

import numpy as np
import ml_dtypes

import concourse.bass as bass
import concourse.mybir as mybir
import concourse.tile as tile
from concourse import bacc
from concourse.bass import ts
from concourse.bass_utils import run_bass_kernel_spmd

B, T, C, H, DK = 4, 2048, 1024, 16, 64
P = 128
NB = T // P          # 16 key blocks
SLOTS = 8            # query blocks per core
SCALE = 1.0 / np.sqrt(DK)
BF16 = mybir.dt.bfloat16
F32 = mybir.dt.float32
F32R = mybir.dt.float32r
EXP = mybir.ActivationFunctionType.Exp

QBLKS = [
    [0, 2, 4, 6, 9, 11, 13, 15],
    [1, 3, 5, 7, 8, 10, 12, 14],
]

_cache = {}


# packed expS layout: row jb stores queries [q0(jb), 1024) where
# q0(jb) = 128*(jb//2); OFF[jb] is the packed column offset.
W_JB = [T // 2 - P * (jb // 2) for jb in range(NB)]
OFF_JB = [0] * NB
for _jb in range(1, NB):
    OFF_JB[_jb] = OFF_JB[_jb - 1] + W_JB[_jb - 1]
NPACK = OFF_JB[-1] + W_JB[-1]  # 9216


def _build():
    nc = bacc.Bacc("TRN2", target_bir_lowering=False, debug=False)

    xT = nc.dram_tensor("xT", [C, T], BF16, kind="ExternalInput").ap()
    xTq = nc.dram_tensor("xTq", [C, SLOTS * P], BF16, kind="ExternalInput").ap()
    wqT = nc.dram_tensor("wqT", [C, C], BF16, kind="ExternalInput").ap()
    wkT = nc.dram_tensor("wkT", [C, C], BF16, kind="ExternalInput").ap()
    wvT = nc.dram_tensor("wvT", [C, C], BF16, kind="ExternalInput").ap()
    woT = nc.dram_tensor("woT", [C, C], BF16, kind="ExternalInput").ap()
    bq = nc.dram_tensor("bq", [P, C // P], F32, kind="ExternalInput").ap()
    bk = nc.dram_tensor("bk", [P, C // P], F32, kind="ExternalInput").ap()
    masks = nc.dram_tensor("masks", [SLOTS, 2, P, P], BF16, kind="ExternalInput").ap()
    ident = nc.dram_tensor("ident", [P, P], BF16, kind="ExternalInput").ap()
    y = nc.dram_tensor("y", [SLOTS * P, C], F32, kind="ExternalOutput").ap()

    CB = C // P  # 8 column blocks of the channel dim

    with tile.TileContext(nc) as tc:
        with (
            tc.tile_pool(name="const", bufs=1) as cpool,
            tc.tile_pool(name="attn", bufs=1) as apool,
        ):
            masks_sb = cpool.tile([P, SLOTS, 2, P], BF16)
            ident_sb = cpool.tile([P, P], BF16)
            bq_sb = cpool.tile([P, CB], F32)
            bk_sb = cpool.tile([P, CB], F32)

            attn_out = apool.tile([P, SLOTS, C], BF16)

            with tc.tile_pool(name="qkv", bufs=1) as qkv:
                qT = qkv.tile([P, CB, SLOTS * P], BF16)
                kT = qkv.tile([P, CB, T], BF16)
                v = qkv.tile([P, NB, H * (DK + 1)], BF16)
                vg = v[:].rearrange("p t (h e) -> p t h e", e=DK + 1)
                nc.vector.memset(vg[:, :, :, DK : DK + 1], 1.0)

                # ---- Q projection (kb-major, 8 psum banks) ----
                with (
                    tc.tile_pool(name="xq", bufs=1) as xq_pool,
                    tc.tile_pool(name="wq", bufs=1) as wq_pool,
                    tc.tile_pool(name="pq", bufs=1, space="PSUM") as pq,
                ):
                    nc.gpsimd.dma_start(bq_sb[:], bq[:])
                    nc.gpsimd.dma_start(bk_sb[:], bk[:])
                    xq_sb = xq_pool.tile([P, CB, SLOTS * P], BF16)
                    xTq_r = xTq.rearrange("(ko p) t -> p ko t", p=P)
                    nc.gpsimd.dma_start(xq_sb[:, 0, :], xTq_r[:, 0, :])
                    wq_sb = wq_pool.tile([P, CB, C], BF16)
                    nc.gpsimd.dma_start(
                        wq_sb[:, 0:4, :],
                        wqT.rearrange("(ko p) n -> p ko n", p=P)[:, 0:4, :],
                    )
                    for kb in range(1, CB):
                        nc.gpsimd.dma_start(xq_sb[:, kb, :], xTq_r[:, kb, :])
                    nc.gpsimd.dma_start(
                        wq_sb[:, 4:8, :],
                        wqT.rearrange("(ko p) n -> p ko n", p=P)[:, 4:8, :],
                    )
                    for nch in range(2):
                        acc = [
                            pq.tile([P, 512], F32, tag=f"qacc{cb}", name=f"qacc{cb}")
                            for cb in range(CB)
                        ]
                        for kb in range(CB):
                            for cb in range(CB):
                                nc.tensor.matmul(
                                    acc[cb][:],
                                    wq_sb[:, kb, ts(cb, P)],
                                    xq_sb[:, kb, ts(nch, 512)],
                                    start=(kb == 0),
                                    stop=(kb == CB - 1),
                                )
                        for cb in range(CB):
                            nc.vector.tensor_scalar_add(
                                qT[:, cb, ts(nch, 512)], acc[cb][:], bq_sb[:, cb : cb + 1]
                            )

                # ---- fused stream: K-proj / scores / exp / attnV / V-proj ----
                # One head-pair per slot; per-slot PE work ~matches the
                # Scalar exp pace so the PE never idles long enough for the
                # HAM clock gate to re-throttle it to 1.2 GHz.
                with (
                    tc.tile_pool(name="xt", bufs=1) as xt_pool,
                    tc.tile_pool(name="wres", bufs=1) as wres,
                    tc.tile_pool(name="expS", bufs=2) as spool,
                    tc.tile_pool(name="small", bufs=4) as small,
                    tc.tile_pool(name="pp", bufs=2, space="PSUM") as pp,
                    tc.tile_pool(name="ps_s", bufs=2, space="PSUM") as ps_s,
                    tc.tile_pool(name="ps_o", bufs=2, space="PSUM") as ps_o,
                ):
                    xT_sb = xt_pool.tile([P, CB, T], BF16)
                    nc.gpsimd.dma_start(
                        xT_sb[:], xT.rearrange("(ko p) t -> p ko t", p=P)
                    )
                    wk_sb = wres.tile([P, CB, C], BF16, name="wk")
                    nc.gpsimd.dma_start(
                        wk_sb[:], wkT.rearrange("(ko p) n -> p ko n", p=P)
                    )
                    nc.gpsimd.dma_start(
                        masks_sb[:], masks[:].rearrange("s t p q -> p s t q")
                    )
                    nc.gpsimd.dma_start(ident_sb[:], ident[:])
                    wv_sb = wres.tile([P, CB, C], BF16, name="wv")
                    nc.gpsimd.dma_start(
                        wv_sb[:], wvT.rearrange("(ko p) n -> p ko n", p=P)
                    )

                    exp_tiles = {}  # h -> expS tile

                    def emit_kproj(cb):
                        # kT[:, cb, :] for one head pair: 4 chains over kb
                        for nch in range(4):
                            acc = pp.tile([P, 512], F32, tag="pp")
                            for kb in range(CB):
                                nc.tensor.matmul(
                                    acc[:],
                                    wk_sb[:, kb, ts(cb, P)],
                                    xT_sb[:, kb, ts(nch, 512)],
                                    start=(kb == 0),
                                    stop=(kb == CB - 1),
                                )
                            nc.vector.tensor_scalar_add(
                                kT[:, cb, ts(nch, 512)], acc[:], bk_sb[:, cb : cb + 1]
                            )

                    def emit_vchain(tb, dch):
                        acc = pp.tile([P, 512], F32, tag="pp")
                        for kb in range(CB):
                            nc.tensor.matmul(
                                acc[:],
                                xT_sb[:, kb, ts(tb, P)],
                                wv_sb[:, kb, ts(dch, 512)],
                                start=(kb == 0),
                                stop=(kb == CB - 1),
                            )
                        nc.vector.tensor_copy(
                            vg[:, tb, dch * 8 : (dch + 1) * 8, 0:DK],
                            acc[:].rearrange("p (h e) -> p h e", e=DK),
                        )

                    def emit_scores(h):
                        # scores + exp + causal masks for head h
                        hp = (h % 2) * DK
                        cbh = h // 2
                        expS = spool.tile([P, NPACK], BF16, tag="expS")
                        exp_tiles[h] = expS
                        for jb in range(0, NB, 2):
                            w = W_JB[jb]
                            q0 = P * (jb // 2)
                            if jb >= 8:
                                # merged pair: both blocks in one 2-bank tile
                                pss = ps_s.tile([P, SLOTS * P], F32, tag="pss")
                                for i in range(2):
                                    nc.tensor.matmul(
                                        pss[:, 512 * i : 512 * i + w],
                                        kT[hp : hp + DK, cbh, ts(jb + i, P)],
                                        qT[hp : hp + DK, cbh, q0:],
                                        start=True,
                                        stop=True,
                                    )
                                nc.scalar.activation(
                                    expS[
                                        :, OFF_JB[jb] : OFF_JB[jb] + 2 * w
                                    ].rearrange("p (t q) -> p t q", t=2),
                                    pss[:].rearrange("p (t c) -> p t c", t=2)[
                                        :, :, 0:w
                                    ],
                                    EXP,
                                    scale=float(SCALE),
                                )
                            else:
                                # separate 2-bank tile per block, one exp each
                                for i in range(2):
                                    pss = ps_s.tile([P, SLOTS * P], F32, tag="pss")
                                    for a, b in (
                                        ((q0, 512), (512, SLOTS * P))
                                        if q0 < 512
                                        else ((q0, SLOTS * P),)
                                    ):
                                        nc.tensor.matmul(
                                            pss[:, a:b],
                                            kT[hp : hp + DK, cbh, ts(jb + i, P)],
                                            qT[hp : hp + DK, cbh, a:b],
                                            start=True,
                                            stop=True,
                                        )
                                    nc.scalar.activation(
                                        expS[:, OFF_JB[jb + i] : OFF_JB[jb + i] + w],
                                        pss[:, q0:],
                                        EXP,
                                        scale=float(SCALE),
                                    )
                            # causal boundary for slot jb//2 (keys 2s, 2s+1)
                            s = jb // 2
                            blk = expS[:, OFF_JB[jb] : OFF_JB[jb] + 2 * w].rearrange(
                                "p (t q) -> p t q", t=2
                            )[:, :, s * P - q0 : (s + 1) * P - q0]
                            nc.vector.tensor_mul(blk, blk, masks_sb[:, s, :, :])

                    def emit_attnv(h):
                        expS = exp_tiles.pop(h)
                        for g in range(2):  # slot groups 0-3 / 4-7
                            pso = ps_o.tile([P, 4, DK + 1], F32, tag="pso")
                            for si in range(4):
                                s = 4 * g + si
                                q0s = [s * P - P * (jb // 2) for jb in range(NB)]
                                J = 2 * s + 2
                                for jb in range(J):
                                    nc.tensor.matmul(
                                        pso[:, si, :],
                                        expS[
                                            :,
                                            OFF_JB[jb] + q0s[jb] : OFF_JB[jb]
                                            + q0s[jb]
                                            + P,
                                        ],
                                        v[:, jb, h * (DK + 1) : (h + 1) * (DK + 1)],
                                        start=(jb == 0),
                                        stop=(jb == J - 1),
                                    )
                            rec = small.tile([P, 4], F32, tag="rec")
                            nc.vector.reciprocal(
                                rec[:],
                                pso[:, :, DK : DK + 1].rearrange("p s o -> p (s o)"),
                            )
                            for si in range(4):
                                s = 4 * g + si
                                nc.vector.tensor_scalar_mul(
                                    attn_out[:, s, h * DK : (h + 1) * DK],
                                    pso[:, si, 0:DK],
                                    rec[:, si : si + 1],
                                )

                    # slot schedule: keep PE saturated, exp chasing scores,
                    # attnV lagging by one slot, V chains as PE filler.
                    vchains = [(tb, 0) for tb in range(NB)] + [
                        (tb, 1) for tb in range(NB)
                    ]
                    emit_kproj(0)
                    emit_scores(0)
                    for tb, dch in vchains[0:8]:
                        emit_vchain(tb, dch)
                    emit_scores(1)
                    for tb, dch in vchains[8:16]:
                        emit_vchain(tb, dch)
                    vnext = 16
                    for c in range(1, CB):
                        emit_kproj(c)
                        emit_scores(2 * c)
                        emit_attnv(2 * c - 2)
                        emit_scores(2 * c + 1)
                        emit_attnv(2 * c - 1)
                        for tb, dch in vchains[vnext : vnext + 4]:
                            emit_vchain(tb, dch)
                        vnext += 4
                    emit_attnv(H - 2)
                    emit_attnv(H - 1)

            # ---- tail: transpose + output projection ----
            with (
                tc.tile_pool(name="out", bufs=1) as opool,
                tc.tile_pool(name="ps_t", bufs=4, space="PSUM") as ps_t,
                tc.tile_pool(name="ps_y", bufs=2, space="PSUM") as ps_y,
            ):
                woT_sb = opool.tile([P, CB, C], BF16)
                nc.gpsimd.dma_start(
                    woT_sb[:], woT.rearrange("(ko p) n -> p ko n", p=P)
                )
                aT = opool.tile([P, CB, SLOTS * P], BF16)
                for cb in range(CB):
                    for s in range(SLOTS):
                        pst = ps_t.tile([P, P], BF16, tag="ps_t")
                        nc.tensor.transpose(
                            pst[:], attn_out[:, s, ts(cb, P)], ident_sb[:]
                        )
                        nc.vector.tensor_copy(aT[:, cb, ts(s, P)], pst[:])

                y_sb = opool.tile([P, SLOTS, C], F32)
                for tb in range(SLOTS):
                    for nch in range(2):
                        psy = ps_y.tile([P, 512], F32, tag="ps_y")
                        for cbk in range(CB):
                            nc.tensor.matmul(
                                psy[:],
                                aT[:, cbk, ts(tb, P)],
                                woT_sb[:, cbk, ts(nch, 512)],
                                start=(cbk == 0),
                                stop=(cbk == CB - 1),
                            )
                        nc.vector.tensor_copy(y_sb[:, tb, ts(nch, 512)], psy[:])
                    nc.gpsimd.dma_start(
                        y.rearrange("(tb p) c -> p tb c", p=P)[:, tb, :],
                        y_sb[:, tb, :],
                    )

    nc.compile()
    return nc


def _host_inputs(x, mask, Wq, bq_v, Wk, bk_v, Wv, bv_v, Wo, bo_v):
    """Per-core input maps + the host-side output bias correction."""
    f32 = np.float32
    bf16 = ml_dtypes.bfloat16
    wqT = np.ascontiguousarray(np.asarray(Wq, f32).T).astype(bf16)
    wkT = np.ascontiguousarray(np.asarray(Wk, f32).T).astype(bf16)
    wvT = np.ascontiguousarray(np.asarray(Wv, f32).T).astype(bf16)
    woT = np.ascontiguousarray(np.asarray(Wo, f32).T).astype(bf16)
    bq_p = np.ascontiguousarray(np.asarray(bq_v, f32).reshape(C // P, P).T)
    bk_p = np.ascontiguousarray(np.asarray(bk_v, f32).reshape(C // P, P).T)
    identity = np.eye(P, dtype=f32).astype(bf16)
    # exact v/o bias fold: softmax rows sum to 1, so v+bv adds bv to attn out
    bo_eff = (np.asarray(bo_v, f32) + np.asarray(bv_v, f32) @ np.asarray(Wo, f32).T)

    # per-half causal boundary masks for the last two key blocks of each slot
    mask_half = []
    tri = np.tril(np.ones((P, P), f32)).T  # [j, i] = 1 where j <= i
    for half in range(2):
        m = np.zeros((SLOTS, 2, P, P), f32)
        for s in range(SLOTS):
            g = QBLKS[half][s]
            for idx, jb in enumerate((2 * s, 2 * s + 1)):
                if jb < g:
                    m[s, idx] = 1.0
                elif jb == g:
                    m[s, idx] = tri
        mask_half.append(m.astype(bf16))

    xn = np.asarray(x, f32)
    in_maps = []
    for core in range(8):
        b, half = divmod(core, 2)
        xT = np.ascontiguousarray(xn[b].T).astype(bf16)
        qtok = np.concatenate([np.arange(g * P, (g + 1) * P) for g in QBLKS[half]])
        xTq = np.ascontiguousarray(xn[b][qtok].T).astype(bf16)
        in_maps.append(
            {
                "xT": xT,
                "xTq": xTq,
                "wqT": wqT,
                "wkT": wkT,
                "wvT": wvT,
                "woT": woT,
                "bq": bq_p,
                "bk": bk_p,
                "masks": mask_half[half],
                "ident": identity,
            }
        )
    return in_maps, bo_eff


def _run(inputs, trace=False):
    if "nc" not in _cache:
        _cache["nc"] = _build()
    nc = _cache["nc"]
    in_maps, bo_eff = _host_inputs(
        inputs["x"], inputs["mask"],
        inputs["Wq"], inputs["bq"], inputs["Wk"], inputs["bk"],
        inputs["Wv"], inputs["bv"], inputs["Wo"], inputs["bo"],
    )
    res = run_bass_kernel_spmd(nc, in_maps, list(range(8)), trace=trace)
    out = np.empty((B, T, C), np.float32)
    for core in range(8):
        b, half = divmod(core, 2)
        yc = res.results[core]["y"]
        for s, g in enumerate(QBLKS[half]):
            out[b, g * P : (g + 1) * P] = yc[s * P : (s + 1) * P]
    out += bo_eff
    return out, res


def kernel(**inputs):
    out, _ = _run(inputs, trace=False)
    return out



# revision 18
# speedup vs baseline: 1.3739x; 1.0553x over previous
"""Multi-head causal attention (B=4, T=2048, C=1024, H=16) on 8 trn2 cores.

Sharding: data-parallel over batch (4) x sequence-parallel over causal query
blocks (2), zig-zag balanced so all 8 cores run one identical program:
  core = 2*b + half;  half 0 gets query blocks [0,2,4,6,9,11,13,15],
  half 1 gets [1,3,5,7,8,10,12,14].  Slot s (0..7) processes J(s)=2s+2 key
  blocks; causal boundary handled by per-core input masks on the last two.
Each core writes a disjoint [1024, 1024] slice of the output; the host
scatters slices back and adds the (v/o-bias) correction  bo + bv @ Wo.T.
"""

import numpy as np
import ml_dtypes

import concourse.bass as bass
import concourse.mybir as mybir
import concourse.tile as tile
from concourse import bacc
from concourse.bass import ts
from concourse.bass_utils import run_bass_kernel_spmd

B, T, C, H, DK = 4, 2048, 1024, 16, 64
P = 128
NB = T // P          # 16 key blocks
SLOTS = 8            # query blocks per core
SCALE = 1.0 / np.sqrt(DK)
BF16 = mybir.dt.bfloat16
F32 = mybir.dt.float32
F32R = mybir.dt.float32r
EXP = mybir.ActivationFunctionType.Exp

QBLKS = [
    [0, 2, 4, 6, 9, 11, 13, 15],
    [1, 3, 5, 7, 8, 10, 12, 14],
]

_cache = {}


# packed expS layout: row jb stores queries [q0(jb), 1024) where
# q0(jb) = 128*(jb//2); OFF[jb] is the packed column offset.
W_JB = [T // 2 - P * (jb // 2) for jb in range(NB)]
OFF_JB = [0] * NB
for _jb in range(1, NB):
    OFF_JB[_jb] = OFF_JB[_jb - 1] + W_JB[_jb - 1]
NPACK = OFF_JB[-1] + W_JB[-1]  # 9216


def _build():
    nc = bacc.Bacc("TRN2", target_bir_lowering=False, debug=False)

    xT = nc.dram_tensor("xT", [C, T], BF16, kind="ExternalInput").ap()
    xTq = nc.dram_tensor("xTq", [C, SLOTS * P], BF16, kind="ExternalInput").ap()
    wqT = nc.dram_tensor("wqT", [C, C], BF16, kind="ExternalInput").ap()
    wkT = nc.dram_tensor("wkT", [C, C], BF16, kind="ExternalInput").ap()
    wvT = nc.dram_tensor("wvT", [C, C], BF16, kind="ExternalInput").ap()
    woT = nc.dram_tensor("woT", [C, C], BF16, kind="ExternalInput").ap()
    bq = nc.dram_tensor("bq", [P, C // P], F32, kind="ExternalInput").ap()
    bk = nc.dram_tensor("bk", [P, C // P], F32, kind="ExternalInput").ap()
    masks = nc.dram_tensor("masks", [SLOTS, 2, P, P], BF16, kind="ExternalInput").ap()
    ident = nc.dram_tensor("ident", [P, P], BF16, kind="ExternalInput").ap()
    y = nc.dram_tensor("y", [SLOTS * P, C], F32, kind="ExternalOutput").ap()

    CB = C // P  # 8 column blocks of the channel dim

    with tile.TileContext(nc) as tc:
        with (
            tc.tile_pool(name="const", bufs=1) as cpool,
            tc.tile_pool(name="attn", bufs=1) as apool,
        ):
            masks_sb = cpool.tile([P, SLOTS, 2, P], BF16)
            ident_sb = cpool.tile([P, P], BF16)
            bq_sb = cpool.tile([P, CB], F32)
            bk_sb = cpool.tile([P, CB], F32)

            attn_out = apool.tile([P, SLOTS, C], BF16)

            with tc.tile_pool(name="qkv", bufs=1) as qkv:
                qT = qkv.tile([P, CB, SLOTS * P], BF16)
                kT = qkv.tile([P, CB, T], BF16)
                v = qkv.tile([P, NB, H * (DK + 1)], BF16)
                vg = v[:].rearrange("p t (h e) -> p t h e", e=DK + 1)
                nc.vector.memset(vg[:, :, :, DK : DK + 1], 1.0)

                # ---- Q projection (kb-major, 8 psum banks) ----
                with (
                    tc.tile_pool(name="xq", bufs=1) as xq_pool,
                    tc.tile_pool(name="wq", bufs=1) as wq_pool,
                    tc.tile_pool(name="pq", bufs=1, space="PSUM") as pq,
                ):
                    nc.gpsimd.dma_start(bq_sb[:], bq[:])
                    nc.gpsimd.dma_start(bk_sb[:], bk[:])
                    xq_sb = xq_pool.tile([P, CB, SLOTS * P], BF16)
                    xTq_r = xTq.rearrange("(ko p) t -> p ko t", p=P)
                    nc.gpsimd.dma_start(xq_sb[:, 0, :], xTq_r[:, 0, :])
                    wq_sb = wq_pool.tile([P, CB, C], BF16)
                    nc.gpsimd.dma_start(
                        wq_sb[:, 0:4, :],
                        wqT.rearrange("(ko p) n -> p ko n", p=P)[:, 0:4, :],
                    )
                    for kb in range(1, CB):
                        nc.gpsimd.dma_start(xq_sb[:, kb, :], xTq_r[:, kb, :])
                    nc.gpsimd.dma_start(
                        wq_sb[:, 4:8, :],
                        wqT.rearrange("(ko p) n -> p ko n", p=P)[:, 4:8, :],
                    )
                    for nch in range(2):
                        acc = [
                            pq.tile([P, 512], F32, tag=f"qacc{cb}", name=f"qacc{cb}")
                            for cb in range(CB)
                        ]
                        for kb in range(CB):
                            for cb in range(CB):
                                nc.tensor.matmul(
                                    acc[cb][:],
                                    wq_sb[:, kb, ts(cb, P)],
                                    xq_sb[:, kb, ts(nch, 512)],
                                    start=(kb == 0),
                                    stop=(kb == CB - 1),
                                )
                        for cb in range(CB):
                            nc.vector.tensor_scalar_add(
                                qT[:, cb, ts(nch, 512)], acc[cb][:], bq_sb[:, cb : cb + 1]
                            )

                # ---- fused stream: K-proj / scores / exp / attnV / V-proj ----
                # Work is emitted as fine-grained units: score psum-groups
                # (Scalar-paced via exp) woven with K/V projection chains and
                # attnV chains, so the PE never idles long enough for the HAM
                # clock gate to re-throttle it, and the Scalar engine always
                # has the next score tile ready.
                with (
                    tc.tile_pool(name="xt", bufs=1) as xt_pool,
                    tc.tile_pool(name="wres", bufs=1) as wres,
                    tc.tile_pool(name="expS", bufs=2) as spool,
                    tc.tile_pool(name="small", bufs=4) as small,
                    tc.tile_pool(name="pp", bufs=2, space="PSUM") as pp,
                    tc.tile_pool(name="ps_s", bufs=2, space="PSUM") as ps_s,
                    tc.tile_pool(name="ps_o", bufs=2, space="PSUM") as ps_o,
                ):
                    xT_sb = xt_pool.tile([P, CB, T], BF16)
                    wk_sb = wres.tile([P, CB, C], BF16, name="wk")
                    wv_sb = wres.tile([P, CB, C], BF16, name="wv")
                    xT_r = xT.rearrange("(ko p) t -> p ko t", p=P)
                    wk_r = wkT.rearrange("(ko p) n -> p ko n", p=P)
                    wv_r = wvT.rearrange("(ko p) n -> p ko n", p=P)
                    # priority order: K-proj inputs first (PE reaches them
                    # ~15us after Q), then the rest.
                    nc.gpsimd.dma_start(wk_sb[:, :, 0:512], wk_r[:, :, 0:512])
                    nc.gpsimd.dma_start(xT_sb[:, :, 0:512], xT_r[:, :, 0:512])
                    nc.gpsimd.dma_start(xT_sb[:, :, 512:1024], xT_r[:, :, 512:1024])
                    nc.gpsimd.dma_start(wk_sb[:, :, 512:1024], wk_r[:, :, 512:1024])
                    nc.gpsimd.dma_start(
                        xT_sb[:, :, 1024:2048], xT_r[:, :, 1024:2048]
                    )
                    nc.gpsimd.dma_start(
                        masks_sb[:], masks[:].rearrange("s t p q -> p s t q")
                    )
                    nc.gpsimd.dma_start(ident_sb[:], ident[:])
                    nc.gpsimd.dma_start(wv_sb[:, :, 0:512], wv_r[:, :, 0:512])
                    nc.gpsimd.dma_start(wv_sb[:, :, 512:1024], wv_r[:, :, 512:1024])

                    exp_tiles = {}

                    def u_kchain(cb, nch):
                        def emit():
                            acc = pp.tile([P, 512], F32, tag="pp")
                            for kb in range(CB):
                                nc.tensor.matmul(
                                    acc[:],
                                    wk_sb[:, kb, ts(cb, P)],
                                    xT_sb[:, kb, ts(nch, 512)],
                                    start=(kb == 0),
                                    stop=(kb == CB - 1),
                                )
                            nc.vector.tensor_scalar_add(
                                kT[:, cb, ts(nch, 512)], acc[:], bk_sb[:, cb : cb + 1]
                            )
                        return emit

                    def u_vchain(tb, dch):
                        def emit():
                            acc = pp.tile([P, 512], F32, tag="pp")
                            for kb in range(CB):
                                nc.tensor.matmul(
                                    acc[:],
                                    xT_sb[:, kb, ts(tb, P)],
                                    wv_sb[:, kb, ts(dch, 512)],
                                    start=(kb == 0),
                                    stop=(kb == CB - 1),
                                )
                            nc.vector.tensor_copy(
                                vg[:, tb, dch * 8 : (dch + 1) * 8, 0:DK],
                                acc[:].rearrange("p (h e) -> p h e", e=DK),
                            )
                        return emit

                    def u_dummy(tb):
                        # keep-warm filler: PE-only V-chain recompute, no reader
                        def emit():
                            acc = pp.tile([P, 512], F32, tag="pp")
                            for kb in range(CB):
                                nc.tensor.matmul(
                                    acc[:],
                                    xT_sb[:, kb, ts(tb, P)],
                                    wv_sb[:, kb, 0:512],
                                    start=(kb == 0),
                                    stop=(kb == CB - 1),
                                )
                        return emit

                    def u_score(h, jb):
                        # one jb-pair group: matmuls + exp (+ causal mask)
                        hp = (h % 2) * DK
                        cbh = h // 2
                        w = W_JB[jb]
                        q0 = P * (jb // 2)
                        s = jb // 2

                        def emit():
                            expS = exp_tiles[h]
                            if jb >= 8:
                                pss = ps_s.tile([P, SLOTS * P], F32, tag="pss")
                                for i in range(2):
                                    nc.tensor.matmul(
                                        pss[:, 512 * i : 512 * i + w],
                                        kT[hp : hp + DK, cbh, ts(jb + i, P)],
                                        qT[hp : hp + DK, cbh, q0:],
                                        start=True,
                                        stop=True,
                                    )
                                nc.scalar.activation(
                                    expS[
                                        :, OFF_JB[jb] : OFF_JB[jb] + 2 * w
                                    ].rearrange("p (t q) -> p t q", t=2),
                                    pss[:].rearrange("p (t c) -> p t c", t=2)[
                                        :, :, 0:w
                                    ],
                                    EXP,
                                    scale=float(SCALE),
                                )
                            else:
                                for i in range(2):
                                    pss = ps_s.tile([P, SLOTS * P], F32, tag="pss")
                                    for aa, bb in ((q0, 512), (512, SLOTS * P)):
                                        nc.tensor.matmul(
                                            pss[:, aa:bb],
                                            kT[hp : hp + DK, cbh, ts(jb + i, P)],
                                            qT[hp : hp + DK, cbh, aa:bb],
                                            start=True,
                                            stop=True,
                                        )
                                    nc.scalar.activation(
                                        expS[:, OFF_JB[jb + i] : OFF_JB[jb + i] + w],
                                        pss[:, q0:],
                                        EXP,
                                        scale=float(SCALE),
                                    )
                            blk = expS[:, OFF_JB[jb] : OFF_JB[jb] + 2 * w].rearrange(
                                "p (t q) -> p t q", t=2
                            )[:, :, 0:P]
                            nc.vector.tensor_mul(blk, blk, masks_sb[:, s, :, :])
                        return emit

                    def u_attnv(h, g):
                        def emit():
                            expS = exp_tiles[h]
                            pso = ps_o.tile([P, 4, DK + 1], F32, tag="pso")
                            for si in range(4):
                                s = 4 * g + si
                                J = 2 * s + 2
                                for jb in range(J):
                                    o = OFF_JB[jb] + s * P - P * (jb // 2)
                                    nc.tensor.matmul(
                                        pso[:, si, :],
                                        expS[:, o : o + P],
                                        v[:, jb, h * (DK + 1) : (h + 1) * (DK + 1)],
                                        start=(jb == 0),
                                        stop=(jb == J - 1),
                                    )
                            rec = small.tile([P, 4], F32, tag="rec")
                            nc.vector.reciprocal(
                                rec[:],
                                pso[:, :, DK : DK + 1].rearrange("p s o -> p (s o)"),
                            )
                            for si in range(4):
                                s = 4 * g + si
                                nc.vector.tensor_scalar_mul(
                                    attn_out[:, s, h * DK : (h + 1) * DK],
                                    pso[:, si, 0:DK],
                                    rec[:, si : si + 1],
                                )
                        return emit

                    def new_head(h):
                        exp_tiles[h] = spool.tile([P, NPACK], BF16, tag="expS", name="expS")

                    # V chain schedule: all of dch0 in slot 0 (A(0)
                    # needs it at slot 1 head), dch1 spread over slots 1-4;
                    # late slots get keep-warm dummy chains instead.
                    vfill = {0: [u_vchain(tb, 0) for tb in range(NB)]}
                    for c in range(1, 5):
                        vfill[c] = [u_vchain(4 * (c - 1) + i, 1) for i in range(4)]
                    for c in range(5, 8):
                        vfill[c] = [u_dummy(i) for i in range(2 * (c - 5), 2 * (c - 5) + 2)]

                    for c in range(CB):
                        h0, h1 = 2 * c, 2 * c + 1
                        fills = list(vfill.get(c, []))
                        nf = len(fills)
                        fi = 0
                        if c >= 1:
                            u_attnv(h0 - 2, 0)()
                            u_attnv(h0 - 2, 1)()
                        new_head(h0)
                        for p in range(8):  # jb-pair index
                            if p % 2 == 0:
                                u_kchain(c, p // 2)()
                            u_score(h0, 2 * p)()
                            while fi * 16 < (p + 1) * nf:
                                fills[fi]()
                                fi += 1
                        if c >= 1:
                            u_attnv(h0 - 1, 0)()
                            u_attnv(h0 - 1, 1)()
                        new_head(h1)
                        for p in range(8):
                            u_score(h1, 2 * p)()
                            while fi * 16 < (p + 9) * nf:
                                fills[fi]()
                                fi += 1
                        for f in fills[fi:]:
                            f()
                    for g in range(2):
                        u_attnv(H - 2, g)()
                    for g in range(2):
                        u_attnv(H - 1, g)()

            # ---- tail: transpose + output projection ----
            with (
                tc.tile_pool(name="out", bufs=1) as opool,
                tc.tile_pool(name="ps_t", bufs=4, space="PSUM") as ps_t,
                tc.tile_pool(name="ps_y", bufs=2, space="PSUM") as ps_y,
            ):
                woT_sb = opool.tile([P, CB, C], BF16)
                nc.gpsimd.dma_start(
                    woT_sb[:], woT.rearrange("(ko p) n -> p ko n", p=P)
                )
                aT = opool.tile([P, CB, SLOTS * P], BF16)
                for cb in range(CB):
                    for s in range(SLOTS):
                        pst = ps_t.tile([P, P], BF16, tag="ps_t")
                        nc.tensor.transpose(
                            pst[:], attn_out[:, s, ts(cb, P)], ident_sb[:]
                        )
                        nc.vector.tensor_copy(aT[:, cb, ts(s, P)], pst[:])

                y_sb = opool.tile([P, SLOTS, C], F32)
                for tb in range(SLOTS):
                    for nch in range(2):
                        psy = ps_y.tile([P, 512], F32, tag="ps_y")
                        for cbk in range(CB):
                            nc.tensor.matmul(
                                psy[:],
                                aT[:, cbk, ts(tb, P)],
                                woT_sb[:, cbk, ts(nch, 512)],
                                start=(cbk == 0),
                                stop=(cbk == CB - 1),
                            )
                        nc.vector.tensor_copy(y_sb[:, tb, ts(nch, 512)], psy[:])
                    nc.gpsimd.dma_start(
                        y.rearrange("(tb p) c -> p tb c", p=P)[:, tb, :],
                        y_sb[:, tb, :],
                    )

    nc.compile()
    return nc


def _host_inputs(x, mask, Wq, bq_v, Wk, bk_v, Wv, bv_v, Wo, bo_v):
    """Per-core input maps + the host-side output bias correction."""
    f32 = np.float32
    bf16 = ml_dtypes.bfloat16
    wqT = np.ascontiguousarray(np.asarray(Wq, f32).T).astype(bf16)
    wkT = np.ascontiguousarray(np.asarray(Wk, f32).T).astype(bf16)
    wvT = np.ascontiguousarray(np.asarray(Wv, f32).T).astype(bf16)
    woT = np.ascontiguousarray(np.asarray(Wo, f32).T).astype(bf16)
    bq_p = np.ascontiguousarray(np.asarray(bq_v, f32).reshape(C // P, P).T)
    bk_p = np.ascontiguousarray(np.asarray(bk_v, f32).reshape(C // P, P).T)
    identity = np.eye(P, dtype=f32).astype(bf16)
    # exact v/o bias fold: softmax rows sum to 1, so v+bv adds bv to attn out
    bo_eff = (np.asarray(bo_v, f32) + np.asarray(bv_v, f32) @ np.asarray(Wo, f32).T)

    # per-half causal boundary masks for the last two key blocks of each slot
    mask_half = []
    tri = np.tril(np.ones((P, P), f32)).T  # [j, i] = 1 where j <= i
    for half in range(2):
        m = np.zeros((SLOTS, 2, P, P), f32)
        for s in range(SLOTS):
            g = QBLKS[half][s]
            for idx, jb in enumerate((2 * s, 2 * s + 1)):
                if jb < g:
                    m[s, idx] = 1.0
                elif jb == g:
                    m[s, idx] = tri
        mask_half.append(m.astype(bf16))

    xn = np.asarray(x, f32)
    in_maps = []
    for core in range(8):
        b, half = divmod(core, 2)
        xT = np.ascontiguousarray(xn[b].T).astype(bf16)
        qtok = np.concatenate([np.arange(g * P, (g + 1) * P) for g in QBLKS[half]])
        xTq = np.ascontiguousarray(xn[b][qtok].T).astype(bf16)
        in_maps.append(
            {
                "xT": xT,
                "xTq": xTq,
                "wqT": wqT,
                "wkT": wkT,
                "wvT": wvT,
                "woT": woT,
                "bq": bq_p,
                "bk": bk_p,
                "masks": mask_half[half],
                "ident": identity,
            }
        )
    return in_maps, bo_eff


def _run(inputs, trace=False):
    if "nc" not in _cache:
        _cache["nc"] = _build()
    nc = _cache["nc"]
    in_maps, bo_eff = _host_inputs(
        inputs["x"], inputs["mask"],
        inputs["Wq"], inputs["bq"], inputs["Wk"], inputs["bk"],
        inputs["Wv"], inputs["bv"], inputs["Wo"], inputs["bo"],
    )
    res = run_bass_kernel_spmd(nc, in_maps, list(range(8)), trace=trace)
    out = np.empty((B, T, C), np.float32)
    for core in range(8):
        b, half = divmod(core, 2)
        yc = res.results[core]["y"]
        for s, g in enumerate(QBLKS[half]):
            out[b, g * P : (g + 1) * P] = yc[s * P : (s + 1) * P]
    out += bo_eff
    return out, res


def kernel(**inputs):
    out, _ = _run(inputs, trace=False)
    return out



# revision 19
# speedup vs baseline: 1.3833x; 1.0068x over previous
"""Multi-head causal attention (B=4, T=2048, C=1024, H=16) on 8 trn2 cores.

Sharding: data-parallel over batch (4) x sequence-parallel over causal query
blocks (2), zig-zag balanced so all 8 cores run one identical program:
  core = 2*b + half;  half 0 gets query blocks [0,2,4,6,9,11,13,15],
  half 1 gets [1,3,5,7,8,10,12,14].  Slot s (0..7) processes J(s)=2s+2 key
  blocks; causal boundary handled by per-core input masks on the last two.
Each core writes a disjoint [1024, 1024] slice of the output; the host
scatters slices back and adds the (v/o-bias) correction  bo + bv @ Wo.T.
"""

import numpy as np
import ml_dtypes

import concourse.bass as bass
import concourse.mybir as mybir
import concourse.tile as tile
from concourse import bacc
from concourse.bass import ts
from concourse.bass_utils import run_bass_kernel_spmd

B, T, C, H, DK = 4, 2048, 1024, 16, 64
P = 128
NB = T // P          # 16 key blocks
SLOTS = 8            # query blocks per core
SCALE = 1.0 / np.sqrt(DK)
BF16 = mybir.dt.bfloat16
F32 = mybir.dt.float32
F32R = mybir.dt.float32r
EXP = mybir.ActivationFunctionType.Exp

QBLKS = [
    [0, 2, 4, 6, 9, 11, 13, 15],
    [1, 3, 5, 7, 8, 10, 12, 14],
]

_cache = {}


# packed expS layout: row jb stores queries [q0(jb), 1024) where
# q0(jb) = 128*(jb//2); OFF[jb] is the packed column offset.
W_JB = [T // 2 - P * (jb // 2) for jb in range(NB)]
OFF_JB = [0] * NB
for _jb in range(1, NB):
    OFF_JB[_jb] = OFF_JB[_jb - 1] + W_JB[_jb - 1]
NPACK = OFF_JB[-1] + W_JB[-1]  # 9216


def _build():
    nc = bacc.Bacc("TRN2", target_bir_lowering=False, debug=False)

    xT = nc.dram_tensor("xT", [C, T], BF16, kind="ExternalInput").ap()
    xTq = nc.dram_tensor("xTq", [C, SLOTS * P], BF16, kind="ExternalInput").ap()
    wqT = nc.dram_tensor("wqT", [C, C], BF16, kind="ExternalInput").ap()
    wkT = nc.dram_tensor("wkT", [C, C], BF16, kind="ExternalInput").ap()
    wvT = nc.dram_tensor("wvT", [C, C], BF16, kind="ExternalInput").ap()
    woT = nc.dram_tensor("woT", [C, C], BF16, kind="ExternalInput").ap()
    bq = nc.dram_tensor("bq", [P, C // P], F32, kind="ExternalInput").ap()
    bk = nc.dram_tensor("bk", [P, C // P], F32, kind="ExternalInput").ap()
    masks = nc.dram_tensor("masks", [SLOTS, 2, P, P], BF16, kind="ExternalInput").ap()
    ident = nc.dram_tensor("ident", [P, P], BF16, kind="ExternalInput").ap()
    y = nc.dram_tensor("y", [SLOTS * P, C], F32, kind="ExternalOutput").ap()

    CB = C // P  # 8 column blocks of the channel dim

    with tile.TileContext(nc) as tc:
        with (
            tc.tile_pool(name="const", bufs=1) as cpool,
            tc.tile_pool(name="attn", bufs=1) as apool,
        ):
            masks_sb = cpool.tile([P, SLOTS, 2, P], BF16)
            ident_sb = cpool.tile([P, P], BF16)
            bq_sb = cpool.tile([P, CB], F32)
            bk_sb = cpool.tile([P, CB], F32)

            attn_out = apool.tile([P, SLOTS, C], BF16)

            with tc.tile_pool(name="qkv", bufs=1) as qkv:
                qT = qkv.tile([P, CB, SLOTS * P], BF16)
                kT = qkv.tile([P, CB, T], BF16)
                v = qkv.tile([P, NB, H * (DK + 1)], BF16)
                vg = v[:].rearrange("p t (h e) -> p t h e", e=DK + 1)
                nc.vector.memset(vg[:, :, :, DK : DK + 1], 1.0)

                # ---- Q projection (kb-major, 8 psum banks) ----
                with (
                    tc.tile_pool(name="xq", bufs=1) as xq_pool,
                    tc.tile_pool(name="wq", bufs=1) as wq_pool,
                    tc.tile_pool(name="pq", bufs=1, space="PSUM") as pq,
                ):
                    nc.gpsimd.dma_start(bq_sb[:], bq[:])
                    nc.gpsimd.dma_start(bk_sb[:], bk[:])
                    xq_sb = xq_pool.tile([P, CB, SLOTS * P], BF16)
                    xTq_r = xTq.rearrange("(ko p) t -> p ko t", p=P)
                    nc.gpsimd.dma_start(xq_sb[:, 0, :], xTq_r[:, 0, :])
                    wq_sb = wq_pool.tile([P, CB, C], BF16)
                    nc.gpsimd.dma_start(
                        wq_sb[:, 0:4, :],
                        wqT.rearrange("(ko p) n -> p ko n", p=P)[:, 0:4, :],
                    )
                    for kb in range(1, 4):
                        nc.gpsimd.dma_start(xq_sb[:, kb, :], xTq_r[:, kb, :])
                    nc.gpsimd.dma_start(
                        wq_sb[:, 4:8, :],
                        wqT.rearrange("(ko p) n -> p ko n", p=P)[:, 4:8, :],
                    )
                    for kb in range(4, CB):
                        nc.gpsimd.dma_start(xq_sb[:, kb, :], xTq_r[:, kb, :])
                    for nch in range(2):
                        acc = [
                            pq.tile([P, 512], F32, tag=f"qacc{cb}", name=f"qacc{cb}")
                            for cb in range(CB)
                        ]
                        for kb in range(CB):
                            for cb in range(CB):
                                nc.tensor.matmul(
                                    acc[cb][:],
                                    wq_sb[:, kb, ts(cb, P)],
                                    xq_sb[:, kb, ts(nch, 512)],
                                    start=(kb == 0),
                                    stop=(kb == CB - 1),
                                )
                        for cb in range(CB):
                            nc.vector.tensor_scalar_add(
                                qT[:, cb, ts(nch, 512)], acc[cb][:], bq_sb[:, cb : cb + 1]
                            )

                # ---- fused stream: K-proj / scores / exp / attnV / V-proj ----
                # Work is emitted as fine-grained units: score psum-groups
                # (Scalar-paced via exp) woven with K/V projection chains and
                # attnV chains, so the PE never idles long enough for the HAM
                # clock gate to re-throttle it, and the Scalar engine always
                # has the next score tile ready.
                with (
                    tc.tile_pool(name="xt", bufs=1) as xt_pool,
                    tc.tile_pool(name="wres", bufs=1) as wres,
                    tc.tile_pool(name="expS", bufs=2) as spool,
                    tc.tile_pool(name="small", bufs=4) as small,
                    tc.tile_pool(name="pp", bufs=2, space="PSUM") as pp,
                    tc.tile_pool(name="ps_s", bufs=2, space="PSUM") as ps_s,
                    tc.tile_pool(name="ps_o", bufs=2, space="PSUM") as ps_o,
                ):
                    xT_sb = xt_pool.tile([P, CB, T], BF16)
                    wk_sb = wres.tile([P, CB, C], BF16, name="wk")
                    wv_sb = wres.tile([P, CB, C], BF16, name="wv")
                    xT_r = xT.rearrange("(ko p) t -> p ko t", p=P)
                    wk_r = wkT.rearrange("(ko p) n -> p ko n", p=P)
                    wv_r = wvT.rearrange("(ko p) n -> p ko n", p=P)
                    # priority order: K-proj inputs first (PE reaches them
                    # ~15us after Q), then the rest.
                    nc.gpsimd.dma_start(wk_sb[:, :, 0:512], wk_r[:, :, 0:512])
                    nc.gpsimd.dma_start(xT_sb[:, :, 0:512], xT_r[:, :, 0:512])
                    nc.gpsimd.dma_start(xT_sb[:, :, 512:1024], xT_r[:, :, 512:1024])
                    nc.gpsimd.dma_start(wk_sb[:, :, 512:1024], wk_r[:, :, 512:1024])
                    nc.gpsimd.dma_start(
                        xT_sb[:, :, 1024:2048], xT_r[:, :, 1024:2048]
                    )
                    nc.gpsimd.dma_start(
                        masks_sb[:], masks[:].rearrange("s t p q -> p s t q")
                    )
                    nc.gpsimd.dma_start(ident_sb[:], ident[:])
                    nc.gpsimd.dma_start(wv_sb[:, :, 0:512], wv_r[:, :, 0:512])
                    nc.gpsimd.dma_start(wv_sb[:, :, 512:1024], wv_r[:, :, 512:1024])

                    exp_tiles = {}

                    def u_kchain(cb, nch):
                        def emit():
                            acc = pp.tile([P, 512], F32, tag="pp")
                            for kb in range(CB):
                                nc.tensor.matmul(
                                    acc[:],
                                    wk_sb[:, kb, ts(cb, P)],
                                    xT_sb[:, kb, ts(nch, 512)],
                                    start=(kb == 0),
                                    stop=(kb == CB - 1),
                                )
                            nc.vector.tensor_scalar_add(
                                kT[:, cb, ts(nch, 512)], acc[:], bk_sb[:, cb : cb + 1]
                            )
                        return emit

                    def u_vchain(tb, dch):
                        def emit():
                            acc = pp.tile([P, 512], F32, tag="pp")
                            for kb in range(CB):
                                nc.tensor.matmul(
                                    acc[:],
                                    xT_sb[:, kb, ts(tb, P)],
                                    wv_sb[:, kb, ts(dch, 512)],
                                    start=(kb == 0),
                                    stop=(kb == CB - 1),
                                )
                            nc.vector.tensor_copy(
                                vg[:, tb, dch * 8 : (dch + 1) * 8, 0:DK],
                                acc[:].rearrange("p (h e) -> p h e", e=DK),
                            )
                        return emit

                    def u_dummy(tb):
                        # keep-warm filler: PE-only V-chain recompute, no reader
                        def emit():
                            acc = pp.tile([P, 512], F32, tag="pp")
                            for kb in range(CB):
                                nc.tensor.matmul(
                                    acc[:],
                                    xT_sb[:, kb, ts(tb, P)],
                                    wv_sb[:, kb, 0:512],
                                    start=(kb == 0),
                                    stop=(kb == CB - 1),
                                )
                        return emit

                    def u_score(h, jb):
                        # one jb-pair group: matmuls + exp (+ causal mask)
                        hp = (h % 2) * DK
                        cbh = h // 2
                        w = W_JB[jb]
                        q0 = P * (jb // 2)
                        s = jb // 2

                        def emit():
                            expS = exp_tiles[h]
                            if jb >= 8:
                                pss = ps_s.tile([P, SLOTS * P], F32, tag="pss")
                                for i in range(2):
                                    nc.tensor.matmul(
                                        pss[:, 512 * i : 512 * i + w],
                                        kT[hp : hp + DK, cbh, ts(jb + i, P)],
                                        qT[hp : hp + DK, cbh, q0:],
                                        start=True,
                                        stop=True,
                                    )
                                nc.scalar.activation(
                                    expS[
                                        :, OFF_JB[jb] : OFF_JB[jb] + 2 * w
                                    ].rearrange("p (t q) -> p t q", t=2),
                                    pss[:].rearrange("p (t c) -> p t c", t=2)[
                                        :, :, 0:w
                                    ],
                                    EXP,
                                    scale=float(SCALE),
                                )
                            else:
                                for i in range(2):
                                    pss = ps_s.tile([P, SLOTS * P], F32, tag="pss")
                                    for aa, bb in ((q0, 512), (512, SLOTS * P)):
                                        nc.tensor.matmul(
                                            pss[:, aa:bb],
                                            kT[hp : hp + DK, cbh, ts(jb + i, P)],
                                            qT[hp : hp + DK, cbh, aa:bb],
                                            start=True,
                                            stop=True,
                                        )
                                    nc.scalar.activation(
                                        expS[:, OFF_JB[jb + i] : OFF_JB[jb + i] + w],
                                        pss[:, q0:],
                                        EXP,
                                        scale=float(SCALE),
                                    )
                            blk = expS[:, OFF_JB[jb] : OFF_JB[jb] + 2 * w].rearrange(
                                "p (t q) -> p t q", t=2
                            )[:, :, 0:P]
                            nc.vector.tensor_mul(blk, blk, masks_sb[:, s, :, :])
                        return emit

                    def u_attnv(h, g):
                        def emit():
                            expS = exp_tiles[h]
                            pso = ps_o.tile([P, 4, DK + 1], F32, tag="pso")
                            for si in range(4):
                                s = 4 * g + si
                                J = 2 * s + 2
                                for jb in range(J):
                                    o = OFF_JB[jb] + s * P - P * (jb // 2)
                                    nc.tensor.matmul(
                                        pso[:, si, :],
                                        expS[:, o : o + P],
                                        v[:, jb, h * (DK + 1) : (h + 1) * (DK + 1)],
                                        start=(jb == 0),
                                        stop=(jb == J - 1),
                                    )
                            rec = small.tile([P, 4], F32, tag="rec")
                            nc.vector.reciprocal(
                                rec[:],
                                pso[:, :, DK : DK + 1].rearrange("p s o -> p (s o)"),
                            )
                            for si in range(4):
                                s = 4 * g + si
                                nc.vector.tensor_scalar_mul(
                                    attn_out[:, s, h * DK : (h + 1) * DK],
                                    pso[:, si, 0:DK],
                                    rec[:, si : si + 1],
                                )
                        return emit

                    def new_head(h):
                        exp_tiles[h] = spool.tile([P, NPACK], BF16, tag="expS", name="expS")

                    # V chain schedule: all of dch0 in slot 0 (A(0)
                    # needs it at slot 1 head), dch1 spread over slots 1-4;
                    # late slots get keep-warm dummy chains instead.
                    vfill = {0: [u_vchain(tb, 0) for tb in range(NB)]}
                    for c in range(1, 5):
                        vfill[c] = [u_vchain(4 * (c - 1) + i, 1) for i in range(4)]
                    for c in range(5, 8):
                        vfill[c] = [u_dummy(c - 5)]

                    for c in range(CB):
                        h0, h1 = 2 * c, 2 * c + 1
                        fills = list(vfill.get(c, []))
                        nf = len(fills)
                        fi = 0
                        if c >= 1:
                            u_attnv(h0 - 2, 0)()
                            u_attnv(h0 - 2, 1)()
                        new_head(h0)
                        for p in range(8):  # jb-pair index
                            if p % 2 == 0:
                                u_kchain(c, p // 2)()
                            u_score(h0, 2 * p)()
                            while fi * 16 < (p + 1) * nf:
                                fills[fi]()
                                fi += 1
                        if c >= 1:
                            u_attnv(h0 - 1, 0)()
                            u_attnv(h0 - 1, 1)()
                        new_head(h1)
                        for p in range(8):
                            u_score(h1, 2 * p)()
                            while fi * 16 < (p + 9) * nf:
                                fills[fi]()
                                fi += 1
                        for f in fills[fi:]:
                            f()
                    for g in range(2):
                        u_attnv(H - 2, g)()
                    for g in range(2):
                        u_attnv(H - 1, g)()

            # ---- tail: transpose + output projection ----
            with (
                tc.tile_pool(name="out", bufs=1) as opool,
                tc.tile_pool(name="ps_t", bufs=4, space="PSUM") as ps_t,
                tc.tile_pool(name="ps_y", bufs=2, space="PSUM") as ps_y,
            ):
                woT_sb = opool.tile([P, CB, C], BF16)
                woT_r = woT.rearrange("(ko p) n -> p ko n", p=P)
                nc.gpsimd.dma_start(woT_sb[:, :, 0:512], woT_r[:, :, 0:512])
                nc.gpsimd.dma_start(woT_sb[:, :, 512:1024], woT_r[:, :, 512:1024])
                aT = opool.tile([P, CB, SLOTS * P], BF16)
                for s in range(SLOTS):
                    for cb in range(CB):
                        pst = ps_t.tile([P, P], BF16, tag="ps_t")
                        nc.tensor.transpose(
                            pst[:], attn_out[:, s, ts(cb, P)], ident_sb[:]
                        )
                        nc.vector.tensor_copy(aT[:, cb, ts(s, P)], pst[:])

                y_sb = opool.tile([P, SLOTS, C], F32)
                for tb in range(SLOTS):
                    for nch in range(2):
                        psy = ps_y.tile([P, 512], F32, tag="ps_y")
                        for cbk in range(CB):
                            nc.tensor.matmul(
                                psy[:],
                                aT[:, cbk, ts(tb, P)],
                                woT_sb[:, cbk, ts(nch, 512)],
                                start=(cbk == 0),
                                stop=(cbk == CB - 1),
                            )
                        nc.vector.tensor_copy(y_sb[:, tb, ts(nch, 512)], psy[:])
                    nc.gpsimd.dma_start(
                        y.rearrange("(tb p) c -> p tb c", p=P)[:, tb, :],
                        y_sb[:, tb, :],
                    )

    nc.compile()
    return nc


def _host_inputs(x, mask, Wq, bq_v, Wk, bk_v, Wv, bv_v, Wo, bo_v):
    """Per-core input maps + the host-side output bias correction."""
    f32 = np.float32
    bf16 = ml_dtypes.bfloat16
    wqT = np.ascontiguousarray(np.asarray(Wq, f32).T).astype(bf16)
    wkT = np.ascontiguousarray(np.asarray(Wk, f32).T).astype(bf16)
    wvT = np.ascontiguousarray(np.asarray(Wv, f32).T).astype(bf16)
    woT = np.ascontiguousarray(np.asarray(Wo, f32).T).astype(bf16)
    bq_p = np.ascontiguousarray(np.asarray(bq_v, f32).reshape(C // P, P).T)
    bk_p = np.ascontiguousarray(np.asarray(bk_v, f32).reshape(C // P, P).T)
    identity = np.eye(P, dtype=f32).astype(bf16)
    # exact v/o bias fold: softmax rows sum to 1, so v+bv adds bv to attn out
    bo_eff = (np.asarray(bo_v, f32) + np.asarray(bv_v, f32) @ np.asarray(Wo, f32).T)

    # per-half causal boundary masks for the last two key blocks of each slot
    mask_half = []
    tri = np.tril(np.ones((P, P), f32)).T  # [j, i] = 1 where j <= i
    for half in range(2):
        m = np.zeros((SLOTS, 2, P, P), f32)
        for s in range(SLOTS):
            g = QBLKS[half][s]
            for idx, jb in enumerate((2 * s, 2 * s + 1)):
                if jb < g:
                    m[s, idx] = 1.0
                elif jb == g:
                    m[s, idx] = tri
        mask_half.append(m.astype(bf16))

    xn = np.asarray(x, f32)
    in_maps = []
    for core in range(8):
        b, half = divmod(core, 2)
        xT = np.ascontiguousarray(xn[b].T).astype(bf16)
        qtok = np.concatenate([np.arange(g * P, (g + 1) * P) for g in QBLKS[half]])
        xTq = np.ascontiguousarray(xn[b][qtok].T).astype(bf16)
        in_maps.append(
            {
                "xT": xT,
                "xTq": xTq,
                "wqT": wqT,
                "wkT": wkT,
                "wvT": wvT,
                "woT": woT,
                "bq": bq_p,
                "bk": bk_p,
                "masks": mask_half[half],
                "ident": identity,
            }
        )
    return in_maps, bo_eff


def _run(inputs, trace=False):
    if "nc" not in _cache:
        _cache["nc"] = _build()
    nc = _cache["nc"]
    in_maps, bo_eff = _host_inputs(
        inputs["x"], inputs["mask"],
        inputs["Wq"], inputs["bq"], inputs["Wk"], inputs["bk"],
        inputs["Wv"], inputs["bv"], inputs["Wo"], inputs["bo"],
    )
    res = run_bass_kernel_spmd(nc, in_maps, list(range(8)), trace=trace)
    out = np.empty((B, T, C), np.float32)
    for core in range(8):
        b, half = divmod(core, 2)
        yc = res.results[core]["y"]
        for s, g in enumerate(QBLKS[half]):
            out[b, g * P : (g + 1) * P] = yc[s * P : (s + 1) * P]
    out += bo_eff
    return out, res


def kernel(**inputs):
    out, _ = _run(inputs, trace=False)
    return out

